# revision 1
# baseline (speedup 1.0000x reference)
"""DiT block kernel for Trainium2 (Bass/Tile), data-parallel over batch on 8 cores.

Per-core dataflow (one batch element per core; no collectives needed):
  - residual stream X [128 tok, 8, 768] fp32 in SBUF, updated in place
  - LayerNorm token-major (bn_stats/bn_aggr) -> xhat bf16 -> PE-transpose
    (batched per token tile) to feature-major XHT [128 d, 6, 1024 tok] bf16
  - per head-pair (2 heads x 64 hs = 128 partitions): Q then K projected with
    bf16 matmuls on a dedicated psum tag, evicted bf16
  - scoresT per (k-tile, head): bf16 row-located matmuls (head A partitions
    0-63, head B 64-127, auto row tile position) into ping-ponged psum tiles
    -> exp on ACT (scale=1/8 folded in; no max subtraction - logits are O(1)
    by construction) -> bf16
  - exp@V and the softmax denominator (all-ones lhsT) col-packed per head
    pair into one [128, 1024] psum (tile_position=(0, 64) for head B);
    normalize via DVE reciprocal+mul; PE-transpose back (batched, deferred
    one pair for overlap); residual added into X in place
  - FFN: h1 feature-major bf16 per ff tile on ping-ponged psum, Silu on ACT
    -> H2 bf16 resident; W2 cast to bf16 ahead of time (gpsimd); second
    matmul bf16; residual fused into the psum eviction
  - weights stream from HBM in chunks (f32) and are cast to bf16 on
    gpsimd/DVE off the critical path; fp32 accumulation everywhere in PSUM
"""

import os
import sys

import numpy as np

for _p in ("/opt/trn_rl_repo", "/root/.axon_site/_ro/trn_rl_repo"):
    if os.path.isdir(_p) and _p not in sys.path:
        sys.path.insert(0, _p)

import concourse.bass as bass
import concourse.mybir as mybir
import concourse.tile as tile
from concourse import bacc
from concourse.bass_utils import run_bass_kernel_spmd
from concourse.masks import make_identity

F32 = mybir.dt.float32
F32R = mybir.dt.float32r
BF16 = mybir.dt.bfloat16
AF = mybir.ActivationFunctionType
OP = mybir.AluOpType

B, T, TC, D, H, HS, FF = 8, 1024, 768 // 3, 768, 12, 64, 3072
P = 128
NT = T // P      # 8 token tiles
NTC = TC // P    # 2 context token tiles
ND = D // P      # 6 feature tiles
NF = FF // P     # 24 ffn tiles
NP = H // 2      # 6 head pairs
EPS = 1e-5
SCALE = HS ** -0.5

WEIGHT_NAMES = [
    "ln1_w", "ln1_b", "sWq", "sbq", "sWk", "sbk", "sWv", "sbv",
    "ln2_w", "ln2_b", "cWq", "cbq", "cWk", "cbk", "cWv", "cbv",
    "ln3_w", "ln3_b", "W1", "b1", "W2", "b2",
]


def _build(flags):
    nc = bacc.Bacc("TRN2", target_bir_lowering=False, debug=False)

    d_img = nc.dram_tensor("img_embedding", [T, D], F32, kind="ExternalInput")
    d_ctx = nc.dram_tensor("context", [TC, D], F32, kind="ExternalInput")
    dw = {}
    for i in (1, 2, 3):
        dw[f"ln{i}_w"] = nc.dram_tensor(f"ln{i}_w", [D], F32, kind="ExternalInput")
        dw[f"ln{i}_b"] = nc.dram_tensor(f"ln{i}_b", [D], F32, kind="ExternalInput")
    for nm in ["sWq", "sWk", "sWv", "cWq", "cWk", "cWv"]:
        dw[nm] = nc.dram_tensor(nm, [H, D, HS], F32, kind="ExternalInput")
    for nm in ["sbq", "sbk", "sbv", "cbq", "cbk", "cbv"]:
        dw[nm] = nc.dram_tensor(nm, [H, HS], F32, kind="ExternalInput")
    dw["W1"] = nc.dram_tensor("W1", [D, FF], F32, kind="ExternalInput")
    dw["b1"] = nc.dram_tensor("b1", [FF], F32, kind="ExternalInput")
    dw["W2"] = nc.dram_tensor("W2", [FF, D], F32, kind="ExternalInput")
    dw["b2"] = nc.dram_tensor("b2", [D], F32, kind="ExternalInput")
    d_out = nc.dram_tensor("out", [T, D], F32, kind="ExternalOutput")
    out_ap = d_out.ap().rearrange("(n p) d -> p n d", p=P)

    with tile.TileContext(nc) as tc, (
        tc.tile_pool(name="const", bufs=1)
    ) as const, (
        tc.tile_pool(name="resid", bufs=1)
    ) as resid, (
        tc.tile_pool(name="wpool", bufs=2)
    ) as wpool, (
        tc.tile_pool(name="big", bufs=1)
    ) as big, (
        tc.tile_pool(name="small", bufs=2)
    ) as small, (
        tc.tile_pool(name="stats", bufs=3)
    ) as stats, (
        tc.tile_pool(name="ps", bufs=1, space="PSUM")
    ) as ps:

        # ---- constants ---------------------------------------------------
        idb = const.tile([P, P], BF16)
        make_identity(nc, idb)
        ones_bf = const.tile([P, HS], BF16)
        nc.vector.memset(ones_bf[:], 1.0)
        eps_t = const.tile([P, 1], F32)
        nc.vector.memset(eps_t[:], EPS)

        def bcast_row(dram_ap, n):
            t = const.tile([P, n], F32)
            src = bass.AP(tensor=dram_ap.tensor, offset=dram_ap.offset,
                          ap=[[0, P]] + list(dram_ap.ap))
            nc.gpsimd.dma_start(t[:], src)
            return t

        ln_w_t, ln_b_t = {}, {}
        for i in (1, 2, 3):
            if not flags[f"ln{i}_w_triv"]:
                ln_w_t[i] = bcast_row(dw[f"ln{i}_w"].ap(), D)
            if not flags[f"ln{i}_b_triv"]:
                ln_b_t[i] = bcast_row(dw[f"ln{i}_b"].ap(), D)
        b2_t = None if flags["b2_zero"] else bcast_row(dw["b2"].ap(), D)

        def pair_bias(nm):
            t = const.tile([P, NP], F32)
            nc.sync.dma_start(
                t[:], dw[nm].ap().rearrange("(g i) e -> (i e) g", i=2))
            return t

        sbq_t = None if flags["sbq_zero"] else pair_bias("sbq")
        sbk_t = None if flags["sbk_zero"] else pair_bias("sbk")
        cbq_t = None if flags["cbq_zero"] else pair_bias("cbq")
        cbk_t = None if flags["cbk_zero"] else pair_bias("cbk")
        sbv_t = None if flags["sbv_zero"] else bcast_row(
            dw["sbv"].ap().rearrange("h e -> (h e)"), D)
        cbv_t = None if flags["cbv_zero"] else bcast_row(
            dw["cbv"].ap().rearrange("h e -> (h e)"), D)
        b1_t = None
        if not flags["b1_zero"]:
            b1_t = const.tile([P, NF], F32)
            nc.sync.dma_start(b1_t[:], dw["b1"].ap().rearrange("(f p) -> p f", p=P))

        # ---- residual stream + context (transposed, bf16) ---------------
        ctxT = resid.tile([P, ND, TC], BF16)
        for t in range(NTC):
            cst = small.tile([P, D], F32, tag="fst")
            nc.sync.dma_start(cst[:], d_ctx.ap().rearrange(
                "(n p) d -> p n d", p=P)[:, t])
            cbf = small.tile([P, D], BF16, tag="xh")
            nc.vector.tensor_copy(cbf[:], cst[:])
            pt = ps.tile([P, D], BF16, tag=("sA" if t % 2 == 0 else "sB"))
            for j in range(ND):
                nc.tensor.transpose(pt[:, j * P:(j + 1) * P],
                                    cbf[:, j * P:(j + 1) * P], idb[:])
            nc.vector.tensor_copy(ctxT[:, :, t * P:(t + 1) * P], pt[:].rearrange(
                "p (j q) -> p j q", q=P))

        X = resid.tile([P, NT, D], F32)
        img_t = d_img.ap().rearrange("(n p) d -> p n d", p=P)
        for t in range(NT):
            nc.sync.dma_start(X[:, t], img_t[:, t])

        # ---- helpers -----------------------------------------------------
        def load_pair_chunk_bf(nm, g):
            """Two heads (2g, 2g+1) of [H, D, HS] -> bf16 [128, ND, 128]."""
            st = wpool.tile([P, ND, P], F32, tag="wst")
            for i in range(2):
                nc.sync.dma_start(
                    st[:, :, i * HS:(i + 1) * HS],
                    dw[nm].ap()[2 * g + i].rearrange("(dt p) e -> p dt e", p=P))
            wb = wpool.tile([P, ND, P], BF16, tag="wbf")
            nc.gpsimd.tensor_copy(wb[:], st[:])
            return wb

        def layernorm_to_T(i, XHT):
            for t in range(NT):
                st = stats.tile([P, 3, 6], F32, tag="bst")
                xg = X[:, t, :].rearrange("p (g d) -> p g d", d=256)
                for g in range(3):
                    nc.vector.bn_stats(st[:, g, :], xg[:, g, :])
                mv = stats.tile([P, 2], F32, tag="mv")
                nc.vector.bn_aggr(mv[:], st[:])
                sd = stats.tile([P, 1], F32, tag="sd")
                nc.scalar.activation(sd[:], mv[:, 1:2], AF.Sqrt, bias=eps_t[:])
                rstd = stats.tile([P, 1], F32, tag="rstd")
                nc.vector.reciprocal(rstd[:], sd[:])
                nmr = stats.tile([P, 1], F32, tag="nmr")
                nc.vector.tensor_scalar(nmr[:], mv[:, 0:1], rstd[:], -1.0,
                                        OP.mult, OP.mult)
                if i in ln_w_t or i in ln_b_t:
                    xf = small.tile([P, D], F32, tag="fst")
                    nc.vector.tensor_scalar(xf[:], X[:, t, :], mv[:, 0:1],
                                            rstd[:], OP.subtract, OP.mult)
                    xh = small.tile([P, D], BF16, tag="xh")
                    if i in ln_w_t and i in ln_b_t:
                        nc.vector.tensor_mul(xf[:], xf[:], ln_w_t[i][:])
                        nc.vector.tensor_tensor(xh[:], xf[:], ln_b_t[i][:], OP.add)
                    elif i in ln_w_t:
                        nc.vector.tensor_tensor(xh[:], xf[:], ln_w_t[i][:], OP.mult)
                    else:
                        nc.vector.tensor_tensor(xh[:], xf[:], ln_b_t[i][:], OP.add)
                else:
                    xh = small.tile([P, D], BF16, tag="xh")
                    nc.scalar.activation(xh[:], X[:, t, :], AF.Identity,
                                         bias=nmr[:], scale=rstd[:])
                pt = ps.tile([P, D], BF16, tag=("sA" if t % 2 == 0 else "sB"))
                for j in range(ND):
                    nc.tensor.transpose(pt[:, j * P:(j + 1) * P],
                                        xh[:, j * P:(j + 1) * P], idb[:])
                nc.vector.tensor_copy(
                    XHT[:, :, t * P:(t + 1) * P],
                    pt[:].rearrange("p (j q) -> p j q", q=P))

        def project_v(nm, XT, n_tok, dest, bias_t):
            """dest [P tok, n_tok//P, D] bf16 token-major V = x @ Wv."""
            wv = big.tile([P, ND, D], BF16, tag="wv")
            for c in range(ND):
                st = wpool.tile([P, ND, P], F32, tag="wst")
                for i in range(2):
                    nc.sync.dma_start(
                        st[:, :, i * HS:(i + 1) * HS],
                        dw[nm].ap()[2 * c + i].rearrange("(dt p) e -> p dt e",
                                                         p=P))
                nc.scalar.copy(wv[:, :, c * P:(c + 1) * P], st[:])
            for t in range(n_tok // P):
                pv = ps.tile([P, D], F32, tag="avs")
                for o, w in ((0, 512), (512, 256)):
                    for dt in range(ND):
                        nc.tensor.matmul(
                            pv[:, o:o + w],
                            XT[:, dt, t * P:(t + 1) * P],
                            wv[:, dt, o:o + w],
                            start=(dt == 0), stop=(dt == ND - 1))
                if bias_t is not None:
                    nc.vector.tensor_tensor(dest[:, t, :], pv[:], bias_t[:],
                                            OP.add)
                else:
                    nc.vector.tensor_copy(dest[:, t, :], pv[:])

        def attention(wq_nm, wk_nm, XT, KXT, n_kv, Vt, qb, kb, pre0=None):
            """Full attention pass; adds output into X in place."""
            nk = n_kv // P
            pending = []

            def flush_attn_out(g, aog):
                pt = ps.tile([P, T], BF16, tag="avs")
                for t in range(NT):
                    nc.tensor.transpose(pt[:, t * P:(t + 1) * P],
                                        aog[:, t * P:(t + 1) * P], idb[:])
                xv = X[:, :, g * P:(g + 1) * P]
                nc.vector.tensor_tensor(
                    xv, pt[:].rearrange("p (t q) -> p t q", q=P), xv, OP.add)

            def do_proj(g):
                if g == 0 and pre0 is not None:
                    wqb, wkb = pre0
                else:
                    wqb = load_pair_chunk_bf(wq_nm, g)
                    wkb = load_pair_chunk_bf(wk_nm, g)
                pq = ps.tile([P, T], F32, tag="pq")
                for c in range(2):
                    for dt in range(ND):
                        nc.tensor.matmul(
                            pq[:, c * 512:(c + 1) * 512],
                            wqb[:, dt, :], XT[:, dt, c * 512:(c + 1) * 512],
                            start=(dt == 0), stop=(dt == ND - 1))
                qg = small.tile([P, T], BF16, tag="qg")
                if qb is not None:
                    nc.vector.tensor_scalar(qg[:], pq[:],
                                            qb[:, g:g + 1], None, OP.add)
                else:
                    nc.vector.tensor_copy(qg[:], pq[:])
                pk = ps.tile([P, n_kv], F32, tag="pq")
                for c in range(max(1, n_kv // 512)):
                    w = min(512, n_kv)
                    for dt in range(ND):
                        nc.tensor.matmul(
                            pk[:, c * w:(c + 1) * w],
                            wkb[:, dt, :], KXT[:, dt, c * w:(c + 1) * w],
                            start=(dt == 0), stop=(dt == ND - 1))
                kg = small.tile([P, n_kv], BF16, tag="kg")
                if kb is not None:
                    nc.vector.tensor_scalar(kg[:], pk[:],
                                            kb[:, g:g + 1], None, OP.add)
                else:
                    nc.vector.tensor_copy(kg[:], pk[:])
                return qg, kg

            carry = do_proj(0)
            for g in range(NP):
                qg, kg = carry

                if nk >= 4:
                    bounds = [2, 2, nk - 4]
                else:
                    bounds = [1, nk - 1, 0]
                tags = ["exphC", "exphB", "exph"]
                offs = [0, bounds[0], bounds[0] + bounds[1]]
                exs = []
                for j in range(3):
                    if bounds[j] > 0:
                        exs.append(big.tile([P, 2, bounds[j], T], BF16,
                                            tag=tags[j], name=f"ex{j}_{g}"))
                    else:
                        exs.append(None)

                def exidx(k):
                    j = 0 if k < offs[1] else (1 if k < offs[2] else 2)
                    return j, k - offs[j]
                for k in range(nk):
                    for i in range(2):
                        sc = ps.tile([P, T], F32,
                                     tag=("sA" if (2 * k + i) % 2 == 0 else "sB"))
                        for c in range(2):
                            nc.tensor.matmul(
                                sc[:, c * 512:(c + 1) * 512],
                                kg[i * HS:(i + 1) * HS, k * P:(k + 1) * P],
                                qg[i * HS:(i + 1) * HS, c * 512:(c + 1) * 512],
                                start=True, stop=True)
                        eh, ek = exidx(k)
                        nc.scalar.activation(exs[eh][:, i, ek, :], sc[:],
                                             AF.Exp, scale=SCALE)

                if g + 1 < NP:
                    carry = do_proj(g + 1)
                while len(pending) > 0:
                    flush_attn_out(*pending.pop(0))

                aog = small.tile([P, T], BF16, tag="aog")
                for c in range(2):
                    po = ps.tile([P, 1024], F32, tag="avs")
                    for k in range(nk):
                        eh, ek = exidx(k)
                        exk = exs[eh]
                        for i in range(2):
                            vcols = slice(g * P + i * HS, g * P + (i + 1) * HS)
                            nc.tensor.matmul(
                                po[i * HS:(i + 1) * HS, 0:512],
                                Vt[:, k, vcols],
                                exk[:, i, ek, c * 512:(c + 1) * 512],
                                start=(k == 0), stop=(k == nk - 1),
                                tile_position=(0, i * HS),
                                skip_group_check=True)
                            nc.tensor.matmul(
                                po[i * HS:(i + 1) * HS, 512:1024],
                                ones_bf[:, :],
                                exk[:, i, ek, c * 512:(c + 1) * 512],
                                start=(k == 0), stop=(k == nk - 1),
                                tile_position=(0, i * HS),
                                skip_group_check=True)
                    rec = small.tile([P, 512], F32, tag="rec")
                    nc.vector.reciprocal(rec[:], po[:, 512:1024])
                    nc.vector.tensor_tensor(aog[:, c * 512:(c + 1) * 512],
                                            po[:, 0:512], rec[:], OP.mult)

                pending.append((g, aog))

            while pending:
                flush_attn_out(*pending.pop(0))

        # =================== self attention ==============================
        XHT = big.tile([P, ND, T], BF16, tag="xht")
        pre0 = (load_pair_chunk_bf("sWq", 0), load_pair_chunk_bf("sWk", 0))
        layernorm_to_T(1, XHT)
        V = big.tile([P, NT, D], BF16, tag="vw2")
        project_v("sWv", XHT, T, V, sbv_t)
        attention("sWq", "sWk", XHT, XHT, T, V, sbq_t, sbk_t, pre0=pre0)

        # =================== cross attention =============================
        XHT2 = big.tile([P, ND, T], BF16, tag="xht")
        layernorm_to_T(2, XHT2)
        Vc = big.tile([P, NTC, D], BF16, tag="vw2")
        project_v("cWv", ctxT, TC, Vc, cbv_t)
        attention("cWq", "cWk", XHT2, ctxT, TC, Vc, cbq_t, cbk_t)

        # =================== FFN =========================================
        XHT3 = big.tile([P, ND, T], BF16, tag="xht")
        layernorm_to_T(3, XHT3)

        W2b = big.tile([P, NF, D], BF16, tag="vw2")
        for f in range(NF):
            st = small.tile([P, D], F32, tag="fst")
            nc.sync.dma_start(st[:], dw["W2"].ap()[f * P:(f + 1) * P, :])
            nc.gpsimd.tensor_copy(W2b[:, f, :], st[:])

        H2 = big.tile([P, NF, T], BF16, tag="exph")
        for fp_ in range(NF // 2):
            st = wpool.tile([P, ND, 2 * P], F32, tag="wst2")
            nc.sync.dma_start(
                st[:], dw["W1"].ap()[:, fp_ * 2 * P:(fp_ + 1) * 2 * P].rearrange(
                    "(dt p) c -> p dt c", p=P))
            w1b = wpool.tile([P, ND, 2 * P], BF16, tag="wbf2")
            nc.vector.tensor_copy(w1b[:], st[:])
            for i in range(2):
                f = fp_ * 2 + i
                ph = ps.tile([P, T], F32, tag=("avs" if i == 0 else "pq"))
                for c in range(2):
                    for dt in range(ND):
                        nc.tensor.matmul(
                            ph[:, c * 512:(c + 1) * 512],
                            w1b[:, dt, i * P:(i + 1) * P],
                            XHT3[:, dt, c * 512:(c + 1) * 512],
                            start=(dt == 0), stop=(dt == ND - 1))
                nc.scalar.activation(
                    H2[:, f, :], ph[:], AF.Silu,
                    bias=(b1_t[:, f:f + 1] if b1_t is not None else 0.0))


        for t in range(NT):
            pf = ps.tile([P, D], F32, tag=("sA" if t % 2 == 0 else "sB"))
            for o, w in ((0, 512), (512, 256)):
                for f in range(NF):
                    nc.tensor.matmul(
                        pf[:, o:o + w],
                        H2[:, f, t * P:(t + 1) * P],
                        W2b[:, f, o:o + w],
                        start=(f == 0), stop=(f == NF - 1))
            ot = small.tile([P, D], F32, tag="ot")
            nc.vector.tensor_tensor(ot[:], pf[:], X[:, t, :], OP.add)
            if b2_t is not None:
                nc.vector.tensor_add(ot[:], ot[:], b2_t[:])
            nc.sync.dma_start(out_ap[:, t], ot[:])

    nc.compile()
    return nc


_CACHE = {}


def _flags_of(inputs):
    f = {}
    for i in (1, 2, 3):
        f[f"ln{i}_w_triv"] = bool(np.all(inputs[f"ln{i}_w"] == 1.0))
        f[f"ln{i}_b_triv"] = bool(np.all(inputs[f"ln{i}_b"] == 0.0))
    for nm in ["sbq", "sbk", "sbv", "cbq", "cbk", "cbv", "b1", "b2"]:
        f[f"{nm}_zero"] = bool(np.all(inputs[nm] == 0.0))
    return f


def kernel(**inputs):
    flags = _flags_of(inputs)
    key = tuple(sorted(flags.items()))
    if key not in _CACHE:
        _CACHE[key] = _build(flags)
    nc = _CACHE[key]

    in_maps = []
    for b in range(B):
        m = {"img_embedding": np.ascontiguousarray(
                 inputs["img_embedding"][b].astype(np.float32)),
             "context": np.ascontiguousarray(
                 inputs["context"][b].astype(np.float32))}
        for nm in WEIGHT_NAMES:
            m[nm] = np.ascontiguousarray(inputs[nm].astype(np.float32))
        in_maps.append(m)

    res = run_bass_kernel_spmd(nc, in_maps, core_ids=list(range(B)))
    return np.stack([res.results[b]["out"] for b in range(B)], axis=0)

